# revision 9
# baseline (speedup 1.0000x reference)
"""Trainium2 Bass kernel for nn_BioRNN: 1000-step leaky-relu RNN scan.

Math per step (reference):
    r_t = relu(h_t)
    y_t = r_t @ W_out_w.T + W_out_b
    h_{t+1} = (1-DT) h_t + DT (x_t @ W_in.T + r_t @ W_rec.T + bias)

Device mapping (per core, batch-sharded 4096 -> 8 x 512):
  - State kept feature-major [H, B] in SBUF as +/- relu parts so ONE
    activation per step per batch-chain produces everything the next step
    needs (relu(h), relu(-h)) plus +/- y staging rows; y = y+ - y- is
    reconstructed on the host.
  - One constant 121x112 matmul per step per chain computes
    [h', -h', y', -y'] from rhs rows [s+(50); s-(50); y+-(12, zero weight);
    x(8); ones(1)].  Decay, input proj, recurrent proj and both biases are
    folded into the weight matrix (ones-row supplies the biases).
  - rhs slots live in a ring at fixed stride; X loads / Y stores move CHUNK
    steps per DMA.  The weight block occupies ring cols 0..111 so that one
    init DMA covers weights + zero state + first X chunk (single wait for
    the first matmul; Matmult supports only 2 sem waits).
"""

import os

import numpy as np

import concourse.bass as bass
import concourse.tile as tile
from concourse import mybir
from concourse.bass_utils import run_bass_kernel_spmd
from concourse.tile import add_dep_helper

DT = 0.1
H, IN, OUT = 50, 8, 6
T, B = 1000, 4096
NCORES = 8
BC = B // NCORES  # 512 batch columns per core

# rhs ring row layout. Compute-engine APs need a 32-aligned partition base, so
# the relu-written block [s+, s-, y+, y-] sits at rows 0..111 and the
# DMA-written block [x, ones] at rows 112..120. The matmul contracts over all
# 121 rows with zero weights on the y rows.
RSP = 0     # relu(h) rows 0..49
RSN = 50    # relu(-h) rows 50..99
RY = 100    # y+/- rows 100..111
RX = 112    # x rows 112..119
RONE = 120  # ones row
NROWS = 121
K = 121     # matmul contraction rows (s+, s-, [y ignored], x, ones)
M = 112     # matmul output rows (h', -h', y', -y')
W0 = 112    # weight block cols 0..111; ring slot columns start here

PSUM_BUFS = 1         # psum slots per chain (1 is enough: the matmul already
                      # waits for the relu via the rhs RAW, same condition as
                      # the PSUM WAR — but 2 decouples ack-latency tails)
CHUNK = 4             # steps per DMA chunk (small: tighter DMA/compute pipelining)
NSEC = 12             # ring sections
NBUF = NSEC * CHUNK   # ring slots

F32 = mybir.dt.float32
F16 = mybir.dt.float16

# batch chains: (col_start, ncols, engine). ACT: (172+FD)/1.2ns from PSUM,
# DVE: (120+FD)/0.96ns. With fp16 matmuls (1 cyc/col vs fp32's 4) the PE is
# no longer the bottleneck; the step period is set by the serial
# mm->relu->mm latency per chain and by ACT/DVE relu occupancy. Fewer,
# larger chains amortize the per-op access overhead; column ranges keep all
# ACT chains first so the Y-DMA engine split stays two contiguous slices.
CHAINS = (
    (0, 87, "act"),
    (87, 87, "act"),
    (174, 169, "vec"),
    (343, 169, "vec"),
)
assert sum(c[1] for c in CHAINS) == BC
# engine column split (act chains first, then vec) — used to split Y-DMAs so
# each carries a single engine-sem wait
ACT_COLS = sum(c[1] for c in CHAINS if c[2] == "act")
assert all(c[2] == "act" for c in CHAINS if c[0] < ACT_COLS)
# PE program order within a step: interleave engines so the in-order PE wait
# queue matches the order relu sems arrive (chain phases settle ~half a
# period apart per engine).
CHAIN_ORDER = (0, 2, 1, 3)


def _build_G(W_in, W_rec, bias, W_out_w, W_out_b):
    G = np.zeros((M, K), np.float32)
    G[0:50, RSP : RSP + 50] = (1.0 - DT) * np.eye(50, dtype=np.float32) + DT * W_rec
    G[0:50, RSN : RSN + 50] = -(1.0 - DT) * np.eye(50, dtype=np.float32)
    G[0:50, RX : RX + 8] = DT * W_in
    G[0:50, RONE] = DT * bias
    G[50:100] = -G[0:50]
    G[100:106, RSP : RSP + 50] = W_out_w
    G[100:106, RONE] = W_out_b
    G[106:112] = -G[100:106]
    return np.ascontiguousarray(G.T.astype(np.float16))  # lhsT [K, M]


def _build_bass(t_steps: int):
    """Build the SPMD Bass program for t_steps."""
    nchx = (t_steps + CHUNK - 1) // CHUNK
    nchy = ((t_steps + 1) + CHUNK - 1) // CHUNK
    nc = bass.Bass("TRN2", debug=False, enable_asserts=False, num_devices=NCORES)
    x_d = nc.dram_tensor("x", [nchx, IN + 1, CHUNK * BC], F16, kind="ExternalInput").ap()
    init_d = nc.dram_tensor(
        "init", [NROWS, W0 + CHUNK * BC], F16, kind="ExternalInput"
    ).ap()
    wb16_d = nc.dram_tensor("wb16", [1, 2], mybir.dt.bfloat16, kind="ExternalInput").ap()
    y_d = nc.dram_tensor("y", [nchy, 12, CHUNK, BC], F16, kind="ExternalOutput").ap()

    with tile.TileContext(nc) as tc:
        with tc.tile_pool(name="ring", bufs=1) as rpool, tc.tile_pool(
            name="psum", bufs=PSUM_BUFS, space="PSUM"
        ) as ppool:
            ring = rpool.tile([NROWS, W0 + NBUF * BC], F16)
            scr = rpool.tile([1, 4096], F32, tag="scr")
            scrw = rpool.tile([1, 2], mybir.dt.bfloat16, tag="scrw")
            scr_idx = [0]
            w_s = ring[0:K, 0:M]

            # init DMAs. Split so y rows are never init-written (a ydma read
            # region must have a single producer class: the relus) and each
            # consumer can absorb one lane tick at a time.
            scrw_i = nc.sync.dma_start(out=scrw[:, :], in_=wb16_d)
            init_a = nc.sync.dma_start(out=ring[:, 0:W0], in_=init_d[:, 0:W0])
            init_b = nc.sync.dma_start(
                out=ring[0:RY, W0 : W0 + CHUNK * BC], in_=init_d[0:RY, W0:]
            )
            gp_dmas = []

            # Steady-state X/Y DMAs go through SWDGE (gpsimd): their trigger
            # instructions live in the POOL engine stream, so POOL carriers
            # can absorb data-dep ticks and each trigger keeps <=1 wait. The
            # few init DMAs stay on HWDGE (<=8, so no lane reuse / no queue
            # waits).
            def xdma(c, first=False):
                sec = W0 + (c % NSEC) * CHUNK * BC
                eng = nc.sync if first else nc.gpsimd
                d = eng.dma_start(
                    out=ring[RX:NROWS, sec : sec + CHUNK * BC], in_=x_d[c]
                )
                if not first:
                    gp_dmas.append(d)
                if c - NSEC in xdmas:
                    # WAW vs the section's previous x-DMA is already implied
                    # transitively by the WAR on the matmuls that read it
                    d.ins.try_remove_dependency(xdmas[c - NSEC].ins.name)
                return d

            def ydma(c):
                # split by engine so each DMA needs one engine-sem tick;
                # chunk 0 skips slot 0 (y rows unwritten, position 0 garbage)
                sec = W0 + (c % NSEC) * CHUNK * BC
                s0 = 1 if c == 0 else 0
                src = ring[RY : RY + 12, sec + s0 * BC : sec + CHUNK * BC]
                src = src.rearrange("p (s b) -> p s b", b=BC)
                da = nc.gpsimd.dma_start(
                    out=y_d[c][:, s0:CHUNK, 0:ACT_COLS], in_=src[:, :, 0:ACT_COLS]
                )
                dv = nc.gpsimd.dma_start(
                    out=y_d[c][:, s0:CHUNK, ACT_COLS:BC], in_=src[:, :, ACT_COLS:BC]
                )
                gp_dmas.extend([da, dv])
                return da, dv

            # Each ISA instruction has ONE sem-wait slot (Matmult two via its
            # LDWEIGHTS). Tile emits a wait for every dep whose tick the
            # engine hasn't observed, so any op with >1 foreign producer
            # over-fills the slot. Wait-carriers fix this: tiny same-engine
            # ops that sync-depend on a DMA, advancing the engine's observed
            # tick so the real relu needs only its matmul wait.
            prev_carrier = {"act": None, "vec": None, "pe": None, "pool": None}

            def carrier(eng, deps):
                # tiny same-engine op that sync-depends on `deps`, advancing
                # the engine's observed ticks so the next real op needs only
                # its single architectural wait slot
                if eng == "pe":
                    # standalone LDWEIGHTS (bf16, tiny); the next real matmul
                    # reloads its own weights
                    c = nc.tensor.ldweights(scrw[0:1, 0:1])
                elif eng == "pool":
                    i = scr_idx[0]
                    scr_idx[0] += 1
                    assert i < 4096
                    c = nc.gpsimd.memset(scr[0:1, i : i + 1], 0.0)
                else:
                    i = scr_idx[0]
                    scr_idx[0] += 1
                    assert i < 4096
                    if eng == "act":
                        c = nc.scalar.activation(
                            scr[0:1, i : i + 1],
                            scrw[0:1, 0:1],
                            mybir.ActivationFunctionType.Copy,
                            bias=0.0,
                        )
                    else:
                        c = nc.vector.tensor_copy(scr[0:1, i : i + 1], scrw[0:1, 0:1])
                for d in deps:
                    add_dep_helper(c.ins, d.ins, sync=True, reason="wait-carrier")
                if prev_carrier[eng] is not None:
                    add_dep_helper(
                        c.ins, prev_carrier[eng].ins, sync=False, reason="order"
                    )
                prev_carrier[eng] = c
                return c

            def pe_carrier(dep):
                return carrier("pe", [dep])

            xdmas, ydmas = {}, {}
            xdmas[0] = xdma(0, first=True)
            if nchx > 1:
                xdmas[1] = xdma(1, first=True)

            # pre-loop carrier chains: first call absorbs the scrw-DMA tick,
            # later ones one init lane each. First-cycle relus WAW-overwrite
            # the init_b zeros; first matmuls read init_a (weights) + init_b.
            carrier("act", [])
            ca = carrier("act", [init_b])
            carrier("vec", [])
            cv = carrier("vec", [init_b])
            carrier("pe", [])
            carrier("pe", [init_a])
            carrier("pe", [init_b])
            last_relu = {"act": None, "vec": None}
            last_mm = None
            chunk_last = {}

            def after_pool_carrier(d):
                add_dep_helper(
                    d.ins, prev_carrier["pool"].ins, sync=False, reason="order"
                )

            for t in range(t_steps):
                if t % CHUNK == 0:
                    k = t // CHUNK
                    if k >= 1:
                        chunk_last[k - 1] = dict(last_relu)
                    if k >= 1:
                        carrier("pool", [last_relu["act"]])
                        carrier("pool", [last_relu["vec"]])
                        yd = ydma(k - 1)
                        after_pool_carrier(yd[0])
                        after_pool_carrier(yd[1])
                        ydmas[k - 1] = yd
                    if k + 2 < nchx:
                        carrier("pool", [last_mm] if last_mm is not None else [])
                        xdmas[k + 2] = xdma(k + 2)
                        after_pool_carrier(xdmas[k + 2])
                    if k in xdmas:
                        # absorb this chunk's x-DMA lane tick before the
                        # first matmul that reads the fresh x rows
                        pe_carrier(xdmas[k])
                    if k >= 2:
                        # upcoming section-flip relu (t=16k+15) overwrites the
                        # section ydma(k-2) read; absorb those ticks first
                        ca = carrier("act", [ydmas[k - 2][0]])
                        cv = carrier("vec", [ydmas[k - 2][1]])
                    if k >= NSEC:
                        # ring-cycle WAW: this chunk's relus overwrite slots
                        # written NBUF steps ago by chunk k-NSEC's relus; the
                        # assigner emits even same-engine WAW as sem waits, so
                        # absorb each engine's own old tick in a carrier
                        ca = carrier("act", [chunk_last[k - NSEC]["act"]])
                        cv = carrier("vec", [chunk_last[k - NSEC]["vec"]])
                sr = W0 + (t % NBUF) * BC        # slot base this step reads
                sw = W0 + ((t + 1) % NBUF) * BC  # slot base the relu writes
                for ci in CHAIN_ORDER:
                    c0, cn, eng = CHAINS[ci]
                    ps = ppool.tile([M, cn], F32, tag=f"ps{ci}")
                    mm = nc.tensor.matmul(
                        ps[:, :],
                        w_s,
                        ring[0:K, sr + c0 : sr + c0 + cn],
                        start=True,
                        stop=True,
                    )
                    if ci == CHAIN_ORDER[0] and prev_carrier["pe"] is not None:
                        add_dep_helper(
                            mm.ins, prev_carrier["pe"].ins, sync=False, reason="order"
                        )
                    last_mm = mm
                    dst = ring[0:M, sw + c0 : sw + c0 + cn]
                    if eng == "act":
                        r = nc.scalar.activation(
                            dst, ps[:, :], mybir.ActivationFunctionType.Relu
                        )
                        if ca is not None:
                            add_dep_helper(r.ins, ca.ins, sync=False, reason="order")
                    else:
                        r = nc.vector.tensor_scalar_max(dst, ps[:, :], 0.0)
                        if cv is not None:
                            add_dep_helper(r.ins, cv.ins, sync=False, reason="order")
                    last_relu[eng] = r
            carrier("pool", [last_relu["act"]])
            carrier("pool", [last_relu["vec"]])
            for c in range((t_steps + CHUNK - 1) // CHUNK - 1, nchy):
                yd = ydma(c)
                after_pool_carrier(yd[0])
                after_pool_carrier(yd[1])

            # SP-nop chain: one dep each, so the TileContext tail drain (an SP
            # instruction waiting for every proc's final tick) finds all its
            # ticks already observed and stays within its single wait slot
            sinks = [scrw_i, init_a, init_b, xdmas[0]]
            if 1 in xdmas:
                sinks.append(xdmas[1])
            sinks += gp_dmas[-8:]
            sinks += [last_mm, last_relu["act"], last_relu["vec"]]
            if prev_carrier["pool"] is not None:
                sinks.append(prev_carrier["pool"])
            prev_nop = None
            for s in sinks:
                n = nc.sync.nop()
                add_dep_helper(n.ins, s.ins, sync=True, reason="drain-prewait")
                if prev_nop is not None:
                    add_dep_helper(n.ins, prev_nop.ins, sync=False, reason="order")
                prev_nop = n
    return nc


def _prep_x(input_core: np.ndarray, t_steps: int):
    """(T, BC, IN) fp32 -> chunked (nchx, IN+1, CHUNK*BC) fp16 contiguous;
    feature row IN is the constant-ones row used for the folded biases."""
    nchx = (t_steps + CHUNK - 1) // CHUNK
    xt = np.zeros((nchx * CHUNK, IN + 1, BC), np.float16)
    xt[:t_steps, :IN] = input_core.transpose(0, 2, 1)
    xt[:, IN] = 1.0
    xc = xt.reshape(nchx, CHUNK, IN + 1, BC).transpose(0, 2, 1, 3)
    return np.ascontiguousarray(xc.reshape(nchx, IN + 1, CHUNK * BC))


def _prep_init(lhsT: np.ndarray, x_chunk0: np.ndarray):
    """[K, M] weights + [IN+1, CHUNK*BC] first x chunk -> [NROWS, W0+CHUNK*BC]."""
    init = np.zeros((NROWS, W0 + CHUNK * BC), np.float16)
    init[:, 0:W0] = lhsT
    init[RX:NROWS, W0:] = x_chunk0
    return np.ascontiguousarray(init)


def kernel(input_seq, W_in, W_rec, bias, W_out_w, W_out_b):
    input_seq = np.asarray(input_seq, dtype=np.float32)
    lhsT = _build_G(
        np.asarray(W_in, np.float32),
        np.asarray(W_rec, np.float32),
        np.asarray(bias, np.float32),
        np.asarray(W_out_w, np.float32),
        np.asarray(W_out_b, np.float32),
    )
    t_steps = input_seq.shape[0]
    nc = _build_bass(t_steps)
    import ml_dtypes

    wb16 = np.zeros((1, 2), dtype=ml_dtypes.bfloat16)
    in_maps = []
    for c in range(NCORES):
        xc = _prep_x(input_seq[:, c * BC : (c + 1) * BC, :], t_steps)
        in_maps.append({"x": xc, "init": _prep_init(lhsT, xc[0]), "wb16": wb16})
    trace = bool(int(os.environ.get("KERNEL_TRACE", "0")))
    res = None
    last_exc = None
    # Tile scheduling has run-to-run nondeterminism; on a rare bad roll the
    # sem assignment can exceed the 1-wait ISA slot and walrus rejects the
    # build. A fresh rebuild re-rolls the schedule, so retry.
    for attempt in range(3):
        try:
            res = run_bass_kernel_spmd(
                nc, in_maps, core_ids=list(range(NCORES)), trace=trace and attempt == 0
            )
            break
        except (ImportError, ModuleNotFoundError):
            trace = False
            continue
        except Exception as e:  # compile/schedule failure — rebuild and retry
            last_exc = e
            nc = _build_bass(t_steps)
    if res is None:
        raise last_exc
    kernel.last_results = res

    nchy = ((t_steps + 1) + CHUNK - 1) // CHUNK
    outs = []
    for c in range(NCORES):
        y = res.results[c]["y"].astype(np.float32)
        yd = y[:, 0:6] - y[:, 6:12]                # (nchy, 6, CHUNK, BC)
        yd = yd.transpose(0, 2, 3, 1).reshape(nchy * CHUNK, BC, OUT)
        outs.append(yd[1 : t_steps + 1])           # position t+1 holds y_t
    return np.ascontiguousarray(np.concatenate(outs, axis=1))


kernel.last_results = None



# revision 42
# speedup vs baseline: 1.3123x; 1.3123x over previous
"""Trainium2 Bass kernel for nn_BioRNN: 1000-step leaky-relu RNN scan.

Math per step (reference):
    r_t = relu(h_t)
    y_t = r_t @ W_out_w.T + W_out_b
    h_{t+1} = (1-DT) h_t + DT (x_t @ W_in.T + r_t @ W_rec.T + bias)

Device mapping (per core, batch-sharded 4096 -> 8 x 512):
  - State kept feature-major [H, B] in SBUF as +/- relu parts so ONE
    activation per step per batch-chain produces everything the next step
    needs (relu(h), relu(-h)) plus +/- y staging rows; y = y+ - y- is
    reconstructed on the host.
  - One constant 121x112 fp16 matmul per step per chain computes
    [h', -h', y', -y'] from rhs rows [s+(50); s-(50); y+-(12, zero weight);
    x(8); ones(1)].  Decay, input proj, recurrent proj and both biases are
    folded into the weight matrix (ones-row supplies the biases).
  - All SBUF data is fp16 (PSUM stays fp32): fp16 matmuls run at 1 cyc/col
    vs fp32's 4, taking the PE off the critical path.  End-to-end fp16
    rounding over 1000 steps measures rel err ~1.7e-3 (tolerance 2e-2);
    bf16 would fail (~2.2e-2).
  - With the PE fast, the step period is bound by the serial
    mm -> relu -> mm loop per chain: mm sem visibility (max(exec,173)+31ns),
    relu busy ((222+c)/1.2 ACT, (120+c)/0.96 DVE) and relu sem tails
    ((444+c)/1.2 ACT, (240+c)/0.96 DVE).  Chain sizes 80/80 (ACT) and
    176/176 (DVE) equalize every chain's round latency at ~703ns, the
    structural floor for a 2-relu-engine layout (GPSIMD cannot read PSUM,
    so a third relu engine is impossible).
  - rhs slots live in a ring at fixed stride; X loads / Y stores move CHUNK
    steps per DMA.  The weight block occupies ring cols 0..111 so that one
    init DMA covers weights + zero state + first X chunk (single wait for
    the first matmul; Matmult supports only 2 sem waits).
  - X is prefetched 4 chunks ahead so the +900ns DMA-sem propagation never
    lands on the critical path; the per-chunk absorb carriers reference
    ticks that are >= NSEC-1 chunks old so they never head-of-line block
    the relu queues.
"""

import os

import numpy as np

import concourse.bass as bass
import concourse.tile as tile
from concourse import mybir
from concourse.bass_utils import run_bass_kernel_spmd
from concourse.tile import add_dep_helper

DT = 0.1
H, IN, OUT = 50, 8, 6
T, B = 1000, 4096
NCORES = 8
BC = B // NCORES  # 512 batch columns per core

# rhs ring row layout. Compute-engine APs need a 32-aligned partition base, so
# the relu-written block [s+, s-, y+, y-] sits at rows 0..111 and the
# DMA-written block [x, ones] at rows 112..120. The matmul contracts over all
# 121 rows with zero weights on the y rows.
RSP = 0     # relu(h) rows 0..49
RSN = 50    # relu(-h) rows 50..99
RY = 100    # y+/- rows 100..111
RX = 112    # x rows 112..119
RONE = 120  # ones row
NROWS = 121
K = 121     # matmul contraction rows (s+, s-, [y ignored], x, ones)
M = 112     # matmul output rows (h', -h', y', -y')
W0 = 112    # weight block cols 0..111; ring slot columns start here

PSUM_BUFS = 1         # psum slots per chain (1 is enough: the matmul already
                      # waits for the relu via the rhs RAW, same condition as
                      # the PSUM WAR — but 2 decouples ack-latency tails)
CHUNK = 6             # steps per DMA chunk (sweet spot with the merged y-DMA; bigger chunks pay more per boundary re-phase, smaller saturate the pool trigger pipeline)
NSEC = 8              # ring sections
NBUF = NSEC * CHUNK   # ring slots

F32 = mybir.dt.float32
F16 = mybir.dt.float16

# batch chains: (col_start, ncols, engine). ACT: (172+FD)/1.2ns from PSUM,
# DVE: (120+FD)/0.96ns. With fp16 matmuls (1 cyc/col vs fp32's 4) the PE is
# no longer the bottleneck; the step period is set by the serial
# mm->relu->mm latency per chain and by ACT/DVE relu occupancy. Fewer,
# larger chains amortize the per-op access overhead; column ranges keep all
# ACT chains first so the Y-DMA engine split stays two contiguous slices.
CHAINS = (
    (0, 80, "act"),
    (80, 80, "act"),
    (160, 176, "vec"),
    (336, 176, "vec"),
)
assert sum(c[1] for c in CHAINS) == BC
# engine column split (act chains first, then vec) — used to split Y-DMAs so
# each carries a single engine-sem wait
ACT_COLS = sum(c[1] for c in CHAINS if c[2] == "act")
assert all(c[2] == "act" for c in CHAINS if c[0] < ACT_COLS)
# PE program order within a step: interleave engines so the in-order PE wait
# queue matches the order relu sems arrive (chain phases settle ~half a
# period apart per engine).
CHAIN_ORDER = (0, 2, 1, 3)


def _build_G(W_in, W_rec, bias, W_out_w, W_out_b):
    G = np.zeros((M, K), np.float32)
    G[0:50, RSP : RSP + 50] = (1.0 - DT) * np.eye(50, dtype=np.float32) + DT * W_rec
    G[0:50, RSN : RSN + 50] = -(1.0 - DT) * np.eye(50, dtype=np.float32)
    G[0:50, RX : RX + 8] = DT * W_in
    G[0:50, RONE] = DT * bias
    G[50:100] = -G[0:50]
    G[100:106, RSP : RSP + 50] = W_out_w
    G[100:106, RONE] = W_out_b
    G[106:112] = -G[100:106]
    return np.ascontiguousarray(G.T.astype(np.float16))  # lhsT [K, M]


BUILD_MAP = {}  # inst name -> ("mm"|"relu", t, chain_idx); for trace analysis


def _build_bass(t_steps: int):
    """Build the SPMD Bass program for t_steps."""
    BUILD_MAP.clear()
    nchx = (t_steps + CHUNK - 1) // CHUNK
    nchy = ((t_steps + 1) + CHUNK - 1) // CHUNK
    nc = bass.Bass("TRN2", debug=False, enable_asserts=False, num_devices=NCORES)
    x_d = nc.dram_tensor("x", [nchx, IN + 1, CHUNK * BC], F16, kind="ExternalInput").ap()
    init_d = nc.dram_tensor(
        "init", [NROWS, W0 + CHUNK * BC], F16, kind="ExternalInput"
    ).ap()
    wb16_d = nc.dram_tensor("wb16", [1, 2], mybir.dt.bfloat16, kind="ExternalInput").ap()
    y_d = nc.dram_tensor("y", [nchy, 12, CHUNK, BC], F16, kind="ExternalOutput").ap()

    with tile.TileContext(nc) as tc:
        with tc.tile_pool(name="ring", bufs=1) as rpool, tc.tile_pool(
            name="psum", bufs=PSUM_BUFS, space="PSUM"
        ) as ppool:
            ring = rpool.tile([NROWS, W0 + NBUF * BC], F16)
            scr = rpool.tile([1, 4096], F32, tag="scr")
            scrw = rpool.tile([1, 2], mybir.dt.bfloat16, tag="scrw")
            scr_idx = [0]
            w_s = ring[0:K, 0:M]

            # init DMAs. Split so y rows are never init-written (a ydma read
            # region must have a single producer class: the relus) and each
            # consumer can absorb one lane tick at a time.
            scrw_i = nc.sync.dma_start(out=scrw[:, :], in_=wb16_d)
            init_a = nc.sync.dma_start(out=ring[:, 0:W0], in_=init_d[:, 0:W0])
            init_b = nc.sync.dma_start(
                out=ring[0:RY, W0 : W0 + CHUNK * BC], in_=init_d[0:RY, W0:]
            )
            # zero slot-0's y rows: the first matmuls contract over them with
            # zero weights, but 0 x garbage-NaN = NaN, so they must be real
            # zeros.  No ydma ever reads slot 0 (chunk 0 skips it), so this
            # one-slot init adds no producer-class conflict.
            init_c = nc.sync.dma_start(
                out=ring[RY:RX, W0 : W0 + BC], in_=init_d[RY:RX, W0 : W0 + BC]
            )
            gp_dmas = []

            # Steady-state X/Y DMAs go through SWDGE (gpsimd): their trigger
            # instructions live in the POOL engine stream, so POOL carriers
            # can absorb data-dep ticks and each trigger keeps <=1 wait. The
            # few init DMAs stay on HWDGE (<=8, so no lane reuse / no queue
            # waits).
            def xdma(c, first=False):
                sec = W0 + (c % NSEC) * CHUNK * BC
                eng = nc.sync if first else nc.gpsimd
                d = eng.dma_start(
                    out=ring[RX:NROWS, sec : sec + CHUNK * BC], in_=x_d[c]
                )
                BUILD_MAP[d.ins.name] = ("xdma", c, 0)
                if not first:
                    gp_dmas.append(d)
                if c - NSEC in xdmas:
                    # WAW vs the section's previous x-DMA is already implied
                    # transitively by the WAR on the matmuls that read it
                    d.ins.try_remove_dependency(xdmas[c - NSEC].ins.name)
                return d

            def ydma(c):
                # one full-width DMA per chunk (pool carriers pre-observe
                # both relu engines' ticks before the trigger);
                # chunk 0 skips slot 0 (y rows unwritten, position 0 garbage)
                sec = W0 + (c % NSEC) * CHUNK * BC
                s0 = 1 if c == 0 else 0
                src = ring[RY : RY + 12, sec + s0 * BC : sec + CHUNK * BC]
                src = src.rearrange("p (s b) -> p s b", b=BC)
                # one merged DMA: the pool carriers pre-observe both ACT and
                # DVE relu ticks before the trigger, so it needs no waits of
                # its own; one SWDGE generation (~1us on the pool engine)
                # instead of two
                d = nc.gpsimd.dma_start(out=y_d[c][:, s0:CHUNK, :], in_=src)
                BUILD_MAP[d.ins.name] = ("ydma", c, 0)
                gp_dmas.append(d)
                return (d, d)

            # Each ISA instruction has ONE sem-wait slot (Matmult two via its
            # LDWEIGHTS). Tile emits a wait for every dep whose tick the
            # engine hasn't observed, so any op with >1 foreign producer
            # over-fills the slot. Wait-carriers fix this: tiny same-engine
            # ops that sync-depend on a DMA, advancing the engine's observed
            # tick so the real relu needs only its matmul wait.
            prev_carrier = {"act": None, "vec": None, "pe": None, "pool": None}

            def carrier(eng, deps):
                # tiny same-engine op that sync-depends on `deps`, advancing
                # the engine's observed ticks so the next real op needs only
                # its single architectural wait slot
                if eng == "pe":
                    # standalone LDWEIGHTS (bf16, tiny); the next real matmul
                    # reloads its own weights
                    c = nc.tensor.ldweights(scrw[0:1, 0:1])
                elif eng == "pool":
                    i = scr_idx[0]
                    scr_idx[0] += 1
                    assert i < 4096
                    c = nc.gpsimd.memset(scr[0:1, i : i + 1], 0.0)
                else:
                    i = scr_idx[0]
                    scr_idx[0] += 1
                    assert i < 4096
                    if eng == "act":
                        c = nc.scalar.activation(
                            scr[0:1, i : i + 1],
                            scrw[0:1, 0:1],
                            mybir.ActivationFunctionType.Copy,
                            bias=0.0,
                        )
                    else:
                        c = nc.vector.tensor_copy(scr[0:1, i : i + 1], scrw[0:1, 0:1])
                for d in deps:
                    if d is None:
                        continue
                    add_dep_helper(c.ins, d.ins, sync=True, reason="wait-carrier")
                if prev_carrier[eng] is not None:
                    add_dep_helper(
                        c.ins, prev_carrier[eng].ins, sync=False, reason="order"
                    )
                prev_carrier[eng] = c
                return c

            def pe_carrier(dep):
                return carrier("pe", [dep])

            xdmas, ydmas = {}, {}
            for c0_ in range(min(4, nchx)):
                xdmas[c0_] = xdma(c0_, first=True)

            # pre-loop carrier chains: first call absorbs the scrw-DMA tick,
            # later ones one init lane each. First-cycle relus WAW-overwrite
            # the init_b zeros; first matmuls read init_a (weights) + init_b.
            carrier("act", [])
            ca = carrier("act", [init_b])
            carrier("act", [init_c])
            carrier("vec", [])
            cv = carrier("vec", [init_b])
            carrier("vec", [init_c])
            carrier("pe", [])
            carrier("pe", [init_a])
            carrier("pe", [init_b])
            carrier("pe", [init_c])
            # pool must observe init_c's queue tick: the ring-wrap ydma
            # (chunk NSEC) reads slot 0 whose y rows init_c also wrote
            carrier("pool", [init_c])
            last_relu = {"act": None, "vec": None}
            last_mm = None
            chunk_last = {}

            def after_pool_carrier(d):
                add_dep_helper(
                    d.ins, prev_carrier["pool"].ins, sync=False, reason="order"
                )

            for t in range(t_steps):
                if t % CHUNK == 0:
                    k = t // CHUNK
                    if k >= 1:
                        chunk_last[k - 1] = dict(last_relu)
                    if k >= 1:
                        if last_relu["act"] is not None:
                            carrier("pool", [last_relu["act"]])
                        if last_relu["vec"] is not None:
                            carrier("pool", [last_relu["vec"]])
                        yd = ydma(k - 1)
                        for d in yd:
                            if d is not None:
                                after_pool_carrier(d)
                        ydmas[k - 1] = yd
                    if k in xdmas:
                        # absorb this chunk's x-DMA lane tick before the
                        # first matmul that reads the fresh x rows (a bare
                        # Matmult has one wait slot; walrus rejects it
                        # carrying the DMA tick alongside the relu wait)
                        pe_carrier(xdmas[k])
                if t % CHUNK == CHUNK // 2:
                    k = t // CHUNK
                    if k + 4 < nchx and k + 4 >= 4:
                        carrier("pool", [last_mm] if last_mm is not None else [])
                        xdmas[k + 4] = xdma(k + 4)
                        after_pool_carrier(xdmas[k + 4])
                if t % CHUNK == 0 and t > 0:
                    k = t // CHUNK
                    if k + 1 - NSEC in ydmas:
                        # the section-flip relu at this chunk's end overwrites
                        # the slots ydma(k+1-NSEC) read; that DMA is ~NSEC-1
                        # chunks old so the carrier's wait is long satisfied
                        # and never head-of-line blocks the relu queue
                        yda, ydv = ydmas[k + 1 - NSEC]
                        if yda is not None:
                            ca = carrier("act", [yda])
                        if ydv is not None:
                            cv = carrier("vec", [ydv])
                    if k >= NSEC:
                        # ring-cycle WAW: the assigner emits even same-engine
                        # WAW as sem waits, so absorb each engine's own old
                        # tick in a carrier
                        if chunk_last[k - NSEC]["act"] is not None:
                            ca = carrier("act", [chunk_last[k - NSEC]["act"]])
                        if chunk_last[k - NSEC]["vec"] is not None:
                            cv = carrier("vec", [chunk_last[k - NSEC]["vec"]])
                sr = W0 + (t % NBUF) * BC        # slot base this step reads
                sw = W0 + ((t + 1) % NBUF) * BC  # slot base the relu writes
                for ci in CHAIN_ORDER:
                    c0, cn, eng = CHAINS[ci]
                    ps = ppool.tile([M, cn], F32, tag=f"ps{ci}")
                    mm = nc.tensor.matmul(
                        ps[:, :],
                        w_s,
                        ring[0:K, sr + c0 : sr + c0 + cn],
                        start=True,
                        stop=True,
                    )
                    if ci == CHAIN_ORDER[0] and prev_carrier["pe"] is not None:
                        add_dep_helper(
                            mm.ins, prev_carrier["pe"].ins, sync=False, reason="order"
                        )
                    BUILD_MAP[mm.ins.name] = ("mm", t, ci)
                    last_mm = mm
                    dst = ring[0:M, sw + c0 : sw + c0 + cn]
                    if eng == "act":
                        r = nc.scalar.activation(
                            dst, ps[:, :], mybir.ActivationFunctionType.Relu
                        )
                        if ca is not None:
                            add_dep_helper(r.ins, ca.ins, sync=False, reason="order")
                    else:
                        r = nc.vector.tensor_scalar_max(dst, ps[:, :], 0.0)
                        if cv is not None:
                            add_dep_helper(r.ins, cv.ins, sync=False, reason="order")
                    BUILD_MAP[r.ins.name] = ("relu", t, ci)
                    last_relu[eng] = r
            if last_relu["act"] is not None:
                carrier("pool", [last_relu["act"]])
            if last_relu["vec"] is not None:
                carrier("pool", [last_relu["vec"]])
            for c in range(max(0, (t_steps + CHUNK - 1) // CHUNK - 1), nchy):
                yd = ydma(c)
                for d in yd:
                    if d is not None:
                        after_pool_carrier(d)

            # SP-nop chain: one dep each, so the TileContext tail drain (an SP
            # instruction waiting for every proc's final tick) finds all its
            # ticks already observed and stays within its single wait slot
            sinks = [scrw_i, init_a, init_b, init_c]
            for c0_ in range(4):
                if c0_ in xdmas:
                    sinks.append(xdmas[c0_])
            sinks += gp_dmas[-8:]
            sinks += [s for s in (last_mm, last_relu["act"], last_relu["vec"]) if s is not None]
            if prev_carrier["pool"] is not None:
                sinks.append(prev_carrier["pool"])
            prev_nop = None
            for s in sinks:
                n = nc.sync.nop()
                add_dep_helper(n.ins, s.ins, sync=True, reason="drain-prewait")
                if prev_nop is not None:
                    add_dep_helper(n.ins, prev_nop.ins, sync=False, reason="order")
                prev_nop = n
    return nc


def _prep_x(input_core: np.ndarray, t_steps: int):
    """(T, BC, IN) fp32 -> chunked (nchx, IN+1, CHUNK*BC) fp16 contiguous;
    feature row IN is the constant-ones row used for the folded biases."""
    nchx = (t_steps + CHUNK - 1) // CHUNK
    xt = np.zeros((nchx * CHUNK, IN + 1, BC), np.float16)
    xt[:t_steps, :IN] = input_core.transpose(0, 2, 1)
    xt[:, IN] = 1.0
    xc = xt.reshape(nchx, CHUNK, IN + 1, BC).transpose(0, 2, 1, 3)
    return np.ascontiguousarray(xc.reshape(nchx, IN + 1, CHUNK * BC))


def _prep_init(lhsT: np.ndarray, x_chunk0: np.ndarray):
    """[K, M] weights + [IN+1, CHUNK*BC] first x chunk -> [NROWS, W0+CHUNK*BC]."""
    init = np.zeros((NROWS, W0 + CHUNK * BC), np.float16)
    init[:, 0:W0] = lhsT
    init[RX:NROWS, W0:] = x_chunk0
    return np.ascontiguousarray(init)


def kernel(input_seq, W_in, W_rec, bias, W_out_w, W_out_b):
    input_seq = np.asarray(input_seq, dtype=np.float32)
    lhsT = _build_G(
        np.asarray(W_in, np.float32),
        np.asarray(W_rec, np.float32),
        np.asarray(bias, np.float32),
        np.asarray(W_out_w, np.float32),
        np.asarray(W_out_b, np.float32),
    )
    t_steps = input_seq.shape[0]
    nc = _build_bass(t_steps)
    import ml_dtypes

    wb16 = np.zeros((1, 2), dtype=ml_dtypes.bfloat16)
    in_maps = []
    for c in range(NCORES):
        xc = _prep_x(input_seq[:, c * BC : (c + 1) * BC, :], t_steps)
        in_maps.append({"x": xc, "init": _prep_init(lhsT, xc[0]), "wb16": wb16})
    trace = bool(int(os.environ.get("KERNEL_TRACE", "0")))
    res = None
    last_exc = None
    # Tile scheduling has run-to-run nondeterminism; on a rare bad roll the
    # sem assignment can exceed the 1-wait ISA slot and walrus rejects the
    # build. A fresh rebuild re-rolls the schedule, so retry.
    for attempt in range(3):
        try:
            res = run_bass_kernel_spmd(
                nc, in_maps, core_ids=list(range(NCORES)), trace=trace and attempt == 0
            )
            break
        except (ImportError, ModuleNotFoundError):
            trace = False
            continue
        except Exception as e:  # compile/schedule failure — rebuild and retry
            last_exc = e
            nc = _build_bass(t_steps)
    if res is None:
        raise last_exc
    kernel.last_results = res

    nchy = ((t_steps + 1) + CHUNK - 1) // CHUNK
    outs = []
    for c in range(NCORES):
        y = res.results[c]["y"].astype(np.float32)
        yd = y[:, 0:6] - y[:, 6:12]                # (nchy, 6, CHUNK, BC)
        yd = yd.transpose(0, 2, 3, 1).reshape(nchy * CHUNK, BC, OUT)
        outs.append(yd[1 : t_steps + 1])           # position t+1 holds y_t
    return np.ascontiguousarray(np.concatenate(outs, axis=1))


kernel.last_results = None

